# revision 1
# baseline (speedup 1.0000x reference)
"""Multi-head attention (GQA, 32 q-heads / 8 kv-heads, S=2048, H=4096) on 8
Trainium2 NeuronCores.

Sharding: tensor-parallel across heads. Core c owns kv-head c and q-heads
4c..4c+3 (Wq/Wk/Wv column-sharded, Wo row-sharded). Each core computes a
partial output [S, H]; the host sums the 8 partials.

Per-core dataflow (everything bf16 into the PE, fp32 accumulation):
  A) qT/kT/vT = W.T @ hiddenT  (weights stationary, hiddenT moving)
     + RoPE applied in the transposed [hd, s] layout
     + vT transposed back to natural v[s, hd] via PE-transpose
  B) per q-head: scoresT[j,i] = kT.T @ qT  ->  E = exp(scale*scoresT)
     denom[i] = onesT @ E (PE),  attnT[d,i] = v.T @ E, normalized on DVE
  C) partial_out[s,:] = attnT.T @ Wo_c  (attnT stationary, Wo moving)
"""

import math
import os
import sys

if os.path.isdir("/opt/trn_rl_repo") and "/opt/trn_rl_repo" not in sys.path:
    sys.path.insert(0, "/opt/trn_rl_repo")

import numpy as np
import ml_dtypes

import concourse.bacc as bacc
import concourse.mybir as mybir
from concourse import tile
from concourse.bass_utils import run_bass_kernel_spmd

BF16 = mybir.dt.bfloat16
F32 = mybir.dt.float32
NPBF16 = ml_dtypes.bfloat16

S = 2048
H = 4096
HD = 128
NH = 32
NKV = 8
N_CORES = 8
QH = NH // N_CORES          # q-heads per core = 4
F = QH * HD                 # q feature columns per core = 512
KT = H // 128               # contraction tiles for the projections = 32
ST = S // 128               # 128-row tiles along S = 16
SG = S // 512               # 512-wide groups along S = 4
SCALE = 1.0 / math.sqrt(HD)

_BUILT = {}


def _build(mode: str):
    masked = mode == "generic"
    nc = bacc.Bacc(None, target_bir_lowering=False)

    hT = nc.declare_dram_parameter("hT", [H, S], BF16, isOutput=False)
    wqkv = nc.declare_dram_parameter("wqkv", [H, F + 2 * HD], BF16, isOutput=False)
    wo = nc.declare_dram_parameter("wo", [F, H], BF16, isOutput=False)
    cosT = nc.declare_dram_parameter("cosT", [HD, S], F32, isOutput=False)
    sinTe = nc.declare_dram_parameter("sinTe", [HD, S], F32, isOutput=False)
    eye = nc.declare_dram_parameter("eye", [128, 128], BF16, isOutput=False)
    if masked:
        maskT = nc.declare_dram_parameter("maskT", [S, S], F32, isOutput=False)
    if mode == "causal":
        # four 0/1 diagonal-tile patterns, stacked [4*128, 512]
        m01 = nc.declare_dram_parameter("m01", [4 * 128, 512], BF16, isOutput=False)
    out = nc.declare_dram_parameter("out", [S, H], F32, isOutput=True)

    FW = F + 2 * HD  # 768 weight columns per contraction tile

    with tile.TileContext(nc) as tc:
        with tc.tile_pool(name="persist", bufs=1) as pp:
            # persistent SBUF tensors
            cos_sb = pp.tile([HD, S], F32, tag="cos")
            sin_sb = pp.tile([HD, S], F32, tag="sin")
            eye_sb = pp.tile([128, 128], BF16, tag="eye")
            ones_sb = pp.tile([128, 128], BF16, tag="ones")
            qT_sb = [pp.tile([HD, S], BF16, tag=f"qT{h}", name=f"qT{h}") for h in range(QH)]
            kT_sb = pp.tile([HD, S], BF16, tag="kT")
            v_sb = pp.tile([128, ST * HD], BF16, tag="v")  # block jt: v[jt*128:(jt+1)*128, :]
            aT_sb = [pp.tile([HD, S], BF16, tag=f"aT{h}", name=f"aT{h}") for h in range(QH)]
            # RoPE scratch lives in the persistent pool so phase-B tiles
            # never alias it (an aliased WAR here would serialize phase B
            # behind the whole RoPE tail)
            vt_t = pp.tile([128, 512], BF16, tag="vt")
            ev_t = {
                f: pp.tile([128, 512], F32, tag=f"ev{f}", name=f"ev{f}")
                for f in (QH, 0, 1, 2, 3)
            }
            t1_t = pp.tile([128, 512], F32, tag="t1")
            t2_t = pp.tile([128, 512], F32, tag="t2")
            if mode == "causal":
                m01_sb = pp.tile([128, 4 * 512], BF16, tag="m01")

            nc.gpsimd.memset(ones_sb[:], 1.0)

            # ---------------- Phase A: projections + RoPE ----------------
            with (
                tc.tile_pool(name="phA_sb", bufs=1) as pa,
                tc.tile_pool(name="phA_h", bufs=1) as pah,
                tc.tile_pool(name="phA_ps", bufs=1, space="PSUM") as pap,
                tc.tile_pool(name="phA_pst", bufs=1, space="PSUM") as papt,
            ):
                w_sb = pa.tile([128, KT * FW], BF16, tag="wqkv")
                w_view = w_sb[:].rearrange("p (a f) -> p a f", a=KT)
                w_src = wqkv[:].rearrange("(a p) f -> p a f", p=128)
                CH = 4  # ktiles per DMA chunk
                for sg in range(SG):
                    hc = pah.tile([128, KT * 512], BF16, tag="hc")
                    h_view = hc[:].rearrange("p (a s) -> p a s", a=KT)
                    h_src = hT[:, sg * 512:(sg + 1) * 512].rearrange(
                        "(a p) s -> p a s", p=128
                    )
                    # interleave weight/hidden chunk loads so the PE can
                    # start as soon as the first k-tiles land
                    bounds = ([0, 1, 2, 4] if sg == 0 else []) + list(
                        range(4 if sg == 0 else 0, KT, CH)
                    )[1 if sg == 0 else 0:]
                    bounds = sorted(set(bounds + [KT]))
                    for lo, hi in zip(bounds[:-1], bounds[1:]):
                        csl = slice(lo, hi)
                        if sg == 0:
                            nc.sync.dma_start(w_view[:, csl, :], w_src[:, csl, :])
                        nc.sync.dma_start(h_view[:, csl, :], h_src[:, csl, :])
                        if sg == 0 and lo == 28:
                            # needed only ~40us in; keep off the queue head
                            nc.sync.dma_start(eye_sb[:], eye[:])
                            nc.sync.dma_start(cos_sb[:], cosT[:])
                            nc.sync.dma_start(sin_sb[:], sinTe[:])
                            if mode == "causal":
                                nc.sync.dma_start(
                                    m01_sb[:].rearrange("p (a i) -> p a i", a=4),
                                    m01[:].rearrange("(a p) i -> p a i", p=128),
                                )
                    pss = [
                        pap.tile([128, 512], F32, tag=f"proj{f}", name=f"proj{f}",
                                 bufs=2 if f == 0 else 1)
                        for f in range(QH + 2)
                    ]
                    for k in range(KT):
                        for f in range(QH + 2):
                            nc.tensor.matmul(
                                pss[f][:],
                                w_sb[:, k * FW + f * 128:k * FW + (f + 1) * 128],
                                hc[:, k * 512:(k + 1) * 512],
                                start=(k == 0),
                                stop=(k == KT - 1),
                            )
                    # Evict all six PSUM groups with plain copies first (banks
                    # free fast, split across DVE/ACT); v-transposes and the
                    # k RoPE come before the q RoPEs so phase B can start.
                    sl = slice(sg * 512, (sg + 1) * 512)
                    vt = vt_t
                    nc.vector.tensor_copy(vt[:], pss[QH + 1][:])
                    for f in (QH, 0, 1, 2, 3):
                        ev = ev_t[f]
                        if f in (0, 2):
                            nc.vector.tensor_copy(ev[:], pss[f][:])
                        else:
                            nc.scalar.copy(ev[:], pss[f][:])
                    for b in range(4):
                        jt = sg * 4 + b
                        pst = papt.tile([128, 128], BF16, tag="vtr")
                        nc.tensor.transpose(
                            pst[:], vt[:, b * 128:(b + 1) * 128], eye_sb[:]
                        )
                        cp = nc.vector.tensor_copy if b % 2 == 0 else nc.scalar.copy
                        cp(v_sb[:, jt * HD:(jt + 1) * HD], pst[:])
                    for f in (QH, 0, 1, 2, 3):
                        # RoPE: out[d] = x[d]*cos[d] + x[(d+64)%128]*sinTe[d]
                        ev = ev_t[f]
                        dest = (qT_sb[f] if f < QH else kT_sb)[:, sl]
                        t1 = t1_t
                        t2 = t2_t
                        nc.vector.tensor_mul(t1[:], ev[:], cos_sb[:, sl])
                        # sin table is host-rolled by 64 rows so both SBUF
                        # inputs share a base partition (walrus constraint)
                        nc.vector.tensor_mul(
                            t2[0:64, :], ev[64:128, :], sin_sb[64:128, sl]
                        )
                        nc.vector.tensor_mul(
                            t2[64:128, :], ev[0:64, :], sin_sb[0:64, sl]
                        )
                        nc.vector.tensor_add(dest, t1[:], t2[:])

            # ---------------- Phase B: attention per head ----------------
            with tc.tile_pool(name="late", bufs=1) as pl:
              # Wo is only needed in phase C; issue its load here so the
              # transfer hides under phase B compute
              wo_sb = pl.tile([128, QH * H], BF16, tag="wo")
              nc.sync.dma_start(
                  wo_sb[:].rearrange("p (a o) -> p a o", a=QH),
                  wo[:].rearrange("(a p) o -> p a o", p=128),
              )
              with (
                tc.tile_pool(name="phB_E", bufs=18) as pe_pool,
                tc.tile_pool(name="phB_tmp", bufs=3) as pbt,
                tc.tile_pool(name="phB_m", bufs=3) as pbm,
                tc.tile_pool(name="phB_s", bufs=4, space="PSUM") as pbs,
                tc.tile_pool(name="phB_acc", bufs=2, space="PSUM") as pba,
              ):
                  for h in range(QH):
                      qh = qT_sb[h]
                      et = [pe_pool.tile([128, S], BF16, tag="E", name=f"E{h}_{j}") for j in range(ST)]
                      for ig in range(SG):
                          isl = slice(ig * 512, (ig + 1) * 512)
                          # causal: key tiles past this query block contribute
                          # exactly zero -- skip them entirely
                          jts = list(range(4 * ig + 4)) if mode == "causal" else list(range(ST))
                          for jt in jts:
                              sps = pbs.tile([128, 512], F32, tag="s")
                              nc.tensor.matmul(
                                  sps[:],
                                  kT_sb[:, jt * 128:(jt + 1) * 128],
                                  qh[:, isl],
                                  start=True,
                                  stop=True,
                              )
                              if masked:
                                  # host pre-scales maskT by sqrt(HD):
                                  # exp(SCALE*(scores + maskT)) == softmax logits
                                  mt = pbm.tile([128, 512], F32, tag="mT")
                                  nc.sync.dma_start(
                                      mt[:], maskT[jt * 128:(jt + 1) * 128, isl]
                                  )
                                  sm = pbm.tile([128, 512], F32, tag="sm")
                                  nc.vector.tensor_add(sm[:], sps[:], mt[:])
                                  nc.scalar.activation(
                                      et[jt][:, isl], sm[:],
                                      mybir.ActivationFunctionType.Exp,
                                      scale=SCALE,
                                  )
                              elif mode == "causal" and jt >= 4 * ig:
                                  # diagonal tile: exp then zero the j>i part
                                  p = jt - 4 * ig
                                  etmp = pbm.tile([128, 512], BF16, tag="etmp")
                                  nc.scalar.activation(
                                      etmp[:], sps[:],
                                      mybir.ActivationFunctionType.Exp,
                                      scale=SCALE,
                                  )
                                  nc.vector.tensor_mul(
                                      et[jt][:, isl], etmp[:],
                                      m01_sb[:, p * 512:(p + 1) * 512],
                                  )
                              else:
                                  nc.scalar.activation(
                                      et[jt][:, isl], sps[:],
                                      mybir.ActivationFunctionType.Exp,
                                      scale=SCALE,
                                  )
                          den = pba.tile([128, 512], F32, tag="den")
                          for jt in jts:
                              nc.tensor.matmul(
                                  den[:], ones_sb[:], et[jt][:, isl],
                                  start=(jt == jts[0]), stop=(jt == jts[-1]),
                              )
                          pv = pba.tile([128, 512], F32, tag="pv")
                          for jt in jts:
                              nc.tensor.matmul(
                                  pv[:], v_sb[:, jt * HD:(jt + 1) * HD], et[jt][:, isl],
                                  start=(jt == jts[0]), stop=(jt == jts[-1]),
                              )
                          rc = pbt.tile([128, 512], F32, tag="rc")
                          nc.vector.reciprocal_approx_fast(rc[:], den[:])
                          nc.vector.tensor_mul(aT_sb[h][:, isl], pv[:], rc[:])

              # ---------------- Phase C: output projection ----------------
              with (
                  tc.tile_pool(name="phC_sb", bufs=1) as pc,
                  tc.tile_pool(name="phC_o", bufs=6) as pco,
                  tc.tile_pool(name="phC_ps", bufs=6, space="PSUM") as pcp,
              ):
                  for st in range(ST):
                      ssl = slice(st * 128, (st + 1) * 128)
                      for ho in range(H // 512):
                          po = pcp.tile([128, 512], F32, tag="o")
                          for f4 in range(QH):
                              nc.tensor.matmul(
                                  po[:],
                                  aT_sb[f4][:, ssl],
                                  wo_sb[:, f4 * H + ho * 512:f4 * H + (ho + 1) * 512],
                                  start=(f4 == 0),
                                  stop=(f4 == QH - 1),
                              )
                          ob = pco.tile([128, 512], F32, tag="ob")
                          if ho % 2 == 0:
                              nc.scalar.copy(ob[:], po[:])
                          else:
                              nc.vector.tensor_copy(ob[:], po[:])
                          nc.sync.dma_start(
                              out[ssl, ho * 512:(ho + 1) * 512], ob[:]
                          )

    nc.finalize()
    return nc


def _get_kernel(mode: str):
    if mode not in _BUILT:
        _BUILT[mode] = _build(mode)
    return _BUILT[mode]


def _detect_mode(mask2d):
    if not np.any(mask2d):
        return "nomask"
    neg = mask2d[0, 1]
    if neg <= -1e4 and np.array_equal(
        mask2d, np.triu(np.full((S, S), neg, mask2d.dtype), k=1)
    ):
        return "causal"
    return "generic"


def kernel(hidden_states, position_ids, attention_mask, cos, sin, Wq, Wk, Wv, Wo,
           _collect_exec_info=None):
    hidden_states = np.asarray(hidden_states)
    attention_mask = np.asarray(attention_mask)
    cos = np.asarray(cos)
    sin = np.asarray(sin)
    Wq, Wk, Wv, Wo = (np.asarray(a) for a in (Wq, Wk, Wv, Wo))

    mode = _detect_mode(attention_mask[0, 0])
    masked = mode == "generic"
    nc = _get_kernel(mode)

    hT = np.ascontiguousarray(hidden_states[0].T).astype(NPBF16)
    cosT = np.ascontiguousarray(cos[0].T).astype(np.float32)
    sinTe = np.ascontiguousarray(sin[0].T).astype(np.float32)
    sinTe[:64] = -sinTe[:64]
    sinTe = np.ascontiguousarray(np.roll(sinTe, 64, axis=0))
    eye = np.eye(128, dtype=NPBF16)
    if mode == "causal":
        jj = np.arange(128)[:, None]
        ii = np.arange(512)[None, :]
        m01 = np.concatenate(
            [(128 * p + jj <= ii).astype(NPBF16) for p in range(4)], axis=0
        )

    in_maps = []
    for c in range(N_CORES):
        wqkv = np.concatenate(
            [
                Wq[:, c * F:(c + 1) * F],
                Wk[:, c * HD:(c + 1) * HD],
                Wv[:, c * HD:(c + 1) * HD],
            ],
            axis=1,
        ).astype(NPBF16)
        m = {
            "hT": hT,
            "wqkv": wqkv,
            "wo": Wo[c * F:(c + 1) * F, :].astype(NPBF16),
            "cosT": cosT,
            "sinTe": sinTe,
            "eye": eye,
        }
        if masked:
            m["maskT"] = (
                np.ascontiguousarray(attention_mask[0, 0].T).astype(np.float32)
                * math.sqrt(HD)
            )
        if mode == "causal":
            m["m01"] = m01
        in_maps.append(m)

    trace = _collect_exec_info is not None
    res = run_bass_kernel_spmd(nc, in_maps, list(range(N_CORES)), trace=trace)
    if trace:
        _collect_exec_info["exec_time_ns"] = res.exec_time_ns
        _collect_exec_info["results"] = res

    acc = res.results[0]["out"].astype(np.float64)
    for c in range(1, N_CORES):
        acc += res.results[c]["out"].astype(np.float64)
    return acc.astype(np.float32)[None, :, :]



# revision 12
# speedup vs baseline: 1.0927x; 1.0927x over previous
"""Multi-head attention (GQA, 32 q-heads / 8 kv-heads, S=2048, H=4096) on 8
Trainium2 NeuronCores.

Sharding: tensor-parallel across heads. Core c owns kv-head c and q-heads
4c..4c+3 (Wq/Wk/Wv column-sharded, Wo row-sharded). Each core computes a
partial output [S, H]; the host sums the 8 partials.

Per-core dataflow (everything bf16 into the PE, fp32 accumulation):
  A) qT/kT/vT = W.T @ hiddenT  (weights stationary, hiddenT moving),
     f-outer passes (one PSUM bank per 512-col pass, 3-bank rotation),
     RoPE applied straight out of PSUM in the transposed [hd, s] layout,
     vT transposed back to natural v[s, hd] via PE-transpose.
  B) per (i-group, q-head): scoresT[j,i] = kT.T @ qT -> E = exp(scale*s)
     denom via DVE chain-sum of the 16 E tiles + ONE ones-matmul,
     attnT[d,i] = v.T @ E (PSUM), normalized on DVE into aT.
  C) partial_out[s,:] = attnT.T @ Wo_c, interleaved per i-group into
     phase B so phase C's PE work hides phase B's ACT (exp) work.
"""

import math
import os
import sys

if os.path.isdir("/opt/trn_rl_repo") and "/opt/trn_rl_repo" not in sys.path:
    sys.path.insert(0, "/opt/trn_rl_repo")

import numpy as np
import ml_dtypes

import concourse.bacc as bacc
import concourse.mybir as mybir
from concourse import tile
from concourse.bass_utils import run_bass_kernel_spmd

BF16 = mybir.dt.bfloat16
F32 = mybir.dt.float32
NPBF16 = ml_dtypes.bfloat16

S = 2048
H = 4096
HD = 128
NH = 32
NKV = 8
N_CORES = 8
QH = NH // N_CORES          # q-heads per core = 4
F = QH * HD                 # q feature columns per core = 512
KT = H // 128               # contraction tiles for the projections = 32
ST = S // 128               # 128-row tiles along S = 16
SG = S // 512               # 512-wide groups along S = 4
NF = QH + 2                 # projection passes per s-group: q0..q3, v, k
SCALE = 1.0 / math.sqrt(HD)

_BUILT = {}


def _build(mode: str):
    masked = mode == "generic"
    nc = bacc.Bacc(None, target_bir_lowering=False)

    hT = nc.declare_dram_parameter("hT", [H, S], BF16, isOutput=False)
    # weight layout: per 128-col feature chunk f (q0..q3, k, v), row = f*128
    # + partition, col = ktile*128 + out-col (8KB contiguous rows -> one
    # full-rate DMA per f-pass)
    wqkv = nc.declare_dram_parameter("wqkv", [NF * 128, KT * 128], BF16,
                                     isOutput=False)
    wo = nc.declare_dram_parameter("wo", [F, H], BF16, isOutput=False)
    cosT = nc.declare_dram_parameter("cosT", [HD, S], F32, isOutput=False)
    sinTe = nc.declare_dram_parameter("sinTe", [HD, S], F32, isOutput=False)
    eye = nc.declare_dram_parameter("eye", [128, 128], BF16, isOutput=False)
    if masked:
        maskT = nc.declare_dram_parameter("maskT", [S, S], F32, isOutput=False)
    if mode == "causal":
        # four 0/1 diagonal-tile patterns, stacked [4*128, 512]
        m01 = nc.declare_dram_parameter("m01", [4 * 128, 512], BF16, isOutput=False)
    out = nc.declare_dram_parameter("out", [S, H], F32, isOutput=True)

    # matmul pass order within each s-group: q heads, then v, then k.
    # (v before k so the PE v-transposes can hide under the k pass.)
    F_ORDER = [0, 1, 2, 3, 5, 4]  # logical f: 0..3 = q heads, 4 = k, 5 = v

    with tile.TileContext(nc) as tc:
        with tc.tile_pool(name="persist", bufs=1) as pp:
            cos_sb = pp.tile([HD, S], F32, tag="cos")
            sin_sb = pp.tile([HD, S], F32, tag="sin")
            eye_sb = pp.tile([128, 128], BF16, tag="eye")
            ones_sb = pp.tile([128, 128], BF16, tag="ones")
            qT_sb = [pp.tile([HD, S], BF16, tag=f"qT{h}", name=f"qT{h}") for h in range(QH)]
            kT_sb = pp.tile([HD, S], BF16, tag="kT")
            v_sb = pp.tile([128, ST * HD], BF16, tag="v")
            # aT double-buffered by i-group parity: phase C reads parity p
            # while phase B writes parity 1-p (avoids any WAR coupling)
            aT_sb = [
                [pp.tile([HD, 512], BF16, tag=f"aT{p}_{h}", name=f"aT{p}_{h}")
                 for h in range(QH)]
                for p in range(2)
            ]
            vt_t = pp.tile([128, 512], BF16, tag="vt")
            t1_t = pp.tile([128, 512], F32, tag="t1")
            t2_t = pp.tile([128, 512], F32, tag="t2")
            if mode == "causal":
                m01_sb = pp.tile([128, 4 * 512], BF16, tag="m01")

            nc.gpsimd.memset(ones_sb[:], 1.0)

            # ---------------- Phase A: projections + RoPE ----------------
            with (
                tc.tile_pool(name="phA_w", bufs=1) as paw,
                tc.tile_pool(name="phA_h", bufs=2) as pah,
                tc.tile_pool(name="phA_ps", bufs=3, space="PSUM") as pap,
                tc.tile_pool(name="phA_pst", bufs=1, space="PSUM") as papt,
            ):
                w_sb = paw.tile([128, NF * KT * 128], BF16, tag="wqkv")
                # w_view[p, f, k, c]
                w_view = w_sb[:].rearrange("p (f a c) -> p f a c", f=NF, a=KT)
                w_flat = w_sb[:].rearrange("p (f q) -> p f q", f=NF)
                w_src = wqkv[:].rearrange("(f p) q -> p f q", f=NF)
                CH = 4  # hidden ktiles per DMA chunk
                for sg in range(SG):
                    hc = pah.tile([128, KT * 512], BF16, tag="hc")
                    h_view = hc[:].rearrange("p (a s) -> p a s", a=KT)
                    h_src = hT[:, sg * 512:(sg + 1) * 512].rearrange(
                        "(a p) s -> p a s", p=128
                    )
                    if sg == 0:
                        # queue order: w for pass 0, the whole hidden group
                        # (pass 0 is DMA-paced), cos/sin (needed by the first
                        # RoPE), remaining w passes, then eye/m01
                        nc.sync.dma_start(w_flat[:, 0], w_src[:, 0])
                    for lo in range(0, KT, CH):
                        csl = slice(lo, lo + CH)
                        nc.sync.dma_start(h_view[:, csl, :], h_src[:, csl, :])
                    if sg == 0:
                        nc.sync.dma_start(cos_sb[:], cosT[:])
                        nc.sync.dma_start(sin_sb[:], sinTe[:])
                        for fi in range(1, NF):
                            nc.sync.dma_start(w_flat[:, fi], w_src[:, fi])
                        nc.sync.dma_start(eye_sb[:], eye[:])
                        if mode == "causal":
                            nc.sync.dma_start(
                                m01_sb[:].rearrange("p (a i) -> p a i", a=4),
                                m01[:].rearrange("(a p) i -> p a i", p=128),
                            )
                    sl = slice(sg * 512, (sg + 1) * 512)
                    for f in F_ORDER:
                        ps = pap.tile([128, 512], F32, tag="proj")
                        for k in range(KT):
                            nc.tensor.matmul(
                                ps[:],
                                w_view[:, f, k, :],
                                hc[:, k * 512:(k + 1) * 512],
                                start=(k == 0),
                                stop=(k == KT - 1),
                            )
                        if f == 5:
                            # v: copy PSUM->SBUF (ACT), transposes emitted
                            # after the k pass below
                            nc.scalar.copy(vt_t[:], ps[:])
                        else:
                            # RoPE straight out of PSUM:
                            # dest[d] = ps[d]*cos[d] + ps[(d+64)%128]*sinTe[d]
                            dest = (qT_sb[f] if f < QH else kT_sb)[:, sl]
                            nc.vector.tensor_mul(t1_t[:], ps[:], cos_sb[:, sl])
                            nc.vector.tensor_mul(
                                t2_t[0:64, :], ps[64:128, :], sin_sb[64:128, sl]
                            )
                            nc.vector.tensor_mul(
                                t2_t[64:128, :], ps[0:64, :], sin_sb[0:64, sl]
                            )
                            nc.vector.tensor_add(dest, t1_t[:], t2_t[:])
                    # v transposes (hidden under the k pass's PE stream)
                    for b in range(4):
                        jt = sg * 4 + b
                        pst = papt.tile([128, 128], BF16, tag="vtr")
                        nc.tensor.transpose(
                            pst[:], vt_t[:, b * 128:(b + 1) * 128], eye_sb[:]
                        )
                        nc.scalar.copy(v_sb[:, jt * HD:(jt + 1) * HD], pst[:])

            # ---------------- Phase B + C interleaved ----------------
            with tc.tile_pool(name="late", bufs=1) as pl:
              wo_sb = pl.tile([128, QH * H], BF16, tag="wo")
              nc.sync.dma_start(
                  wo_sb[:].rearrange("p (a o) -> p a o", a=QH),
                  wo[:].rearrange("(a p) o -> p a o", p=128),
              )
              with (
                tc.tile_pool(name="phB_E", bufs=20) as pe_pool,
                tc.tile_pool(name="phB_acc", bufs=3) as pacc,
                tc.tile_pool(name="phB_rc", bufs=3) as prc,
                tc.tile_pool(name="phB_m", bufs=3) as pbm,
                tc.tile_pool(name="phB_s", bufs=4, space="PSUM") as pbs,
                tc.tile_pool(name="phB_pv", bufs=1, space="PSUM") as pbv,
                tc.tile_pool(name="phB_den", bufs=1, space="PSUM") as pbd,
                tc.tile_pool(name="phC_ps", bufs=2, space="PSUM") as pcp,
                tc.tile_pool(name="phC_o", bufs=4) as pco,
              ):
                  def c_block(st):
                      ssl = slice(st * 128, (st + 1) * 128)
                      par = (st // 4) % 2
                      off = (st % 4) * 128
                      for ho in range(H // 512):
                          po = pcp.tile([128, 512], F32, tag="o")
                          for f4 in range(QH):
                              nc.tensor.matmul(
                                  po[:],
                                  aT_sb[par][f4][:, off:off + 128],
                                  wo_sb[:, f4 * H + ho * 512:f4 * H + (ho + 1) * 512],
                                  start=(f4 == 0),
                                  stop=(f4 == QH - 1),
                              )
                          ob = pco.tile([128, 512], F32, tag="ob")
                          if ho % 2 == 0:
                              nc.scalar.copy(ob[:], po[:])
                          else:
                              nc.vector.tensor_copy(ob[:], po[:])
                          nc.sync.dma_start(
                              out[ssl, ho * 512:(ho + 1) * 512], ob[:]
                          )

                  for ig in range(SG):
                      isl = slice(ig * 512, (ig + 1) * 512)
                      par = ig % 2
                      jts = list(range(4 * ig + 4)) if mode == "causal" else list(range(ST))
                      for h in range(QH):
                          qh = qT_sb[h]
                          et = [
                              pe_pool.tile([128, 512], BF16, tag="E",
                                           name=f"E{ig}_{h}_{j}")
                              for j in jts
                          ]
                          for x, jt in enumerate(jts):
                              sps = pbs.tile([128, 512], F32, tag="s")
                              nc.tensor.matmul(
                                  sps[:],
                                  kT_sb[:, jt * 128:(jt + 1) * 128],
                                  qh[:, isl],
                                  start=True,
                                  stop=True,
                              )
                              if masked:
                                  mt = pbm.tile([128, 512], F32, tag="mT")
                                  nc.sync.dma_start(
                                      mt[:], maskT[jt * 128:(jt + 1) * 128, isl]
                                  )
                                  sm = pbm.tile([128, 512], F32, tag="sm")
                                  nc.vector.tensor_add(sm[:], sps[:], mt[:])
                                  nc.scalar.activation(
                                      et[x][:], sm[:],
                                      mybir.ActivationFunctionType.Exp,
                                      scale=SCALE,
                                  )
                              elif mode == "causal" and jt >= 4 * ig:
                                  p = jt - 4 * ig
                                  etmp = pbm.tile([128, 512], BF16, tag="etmp")
                                  nc.scalar.activation(
                                      etmp[:], sps[:],
                                      mybir.ActivationFunctionType.Exp,
                                      scale=SCALE,
                                  )
                                  nc.vector.tensor_mul(
                                      et[x][:], etmp[:],
                                      m01_sb[:, p * 512:(p + 1) * 512],
                                  )
                              else:
                                  nc.scalar.activation(
                                      et[x][:], sps[:],
                                      mybir.ActivationFunctionType.Exp,
                                      scale=SCALE,
                                  )
                          # denominator: chain-sum the E tiles on DVE (bf16),
                          # finish with a single ones-matmul
                          acc = pacc.tile([128, 512], BF16, tag="acc")
                          nc.vector.tensor_add(acc[:], et[0][:], et[1][:])
                          for x in range(2, len(jts)):
                              nxt = pacc.tile([128, 512], BF16, tag="acc")
                              nc.vector.tensor_add(nxt[:], acc[:], et[x][:])
                              acc = nxt
                          pv = pbv.tile([128, 512], F32, tag="pv")
                          for x, jt in enumerate(jts):
                              nc.tensor.matmul(
                                  pv[:], v_sb[:, jt * HD:(jt + 1) * HD], et[x][:],
                                  start=(x == 0), stop=(x == len(jts) - 1),
                              )
                          den = pbd.tile([128, 512], F32, tag="den")
                          nc.tensor.matmul(
                              den[:], ones_sb[:], acc[:], start=True, stop=True
                          )
                          rc = prc.tile([128, 512], F32, tag="rc")
                          nc.vector.reciprocal_approx_fast(rc[:], den[:])
                          nc.vector.tensor_mul(aT_sb[par][h][:], pv[:], rc[:])
                          if ig >= 1:
                              c_block(4 * (ig - 1) + h)
                  for st in range(4 * (SG - 1), ST):
                      c_block(st)

    nc.finalize()
    return nc


def _get_kernel(mode: str):
    if mode not in _BUILT:
        _BUILT[mode] = _build(mode)
    return _BUILT[mode]


def _detect_mode(mask2d):
    if not np.any(mask2d):
        return "nomask"
    neg = mask2d[0, 1]
    if neg <= -1e4 and np.array_equal(
        mask2d, np.triu(np.full((S, S), neg, mask2d.dtype), k=1)
    ):
        return "causal"
    return "generic"


def kernel(hidden_states, position_ids, attention_mask, cos, sin, Wq, Wk, Wv, Wo,
           _collect_exec_info=None):
    hidden_states = np.asarray(hidden_states)
    attention_mask = np.asarray(attention_mask)
    cos = np.asarray(cos)
    sin = np.asarray(sin)
    Wq, Wk, Wv, Wo = (np.asarray(a) for a in (Wq, Wk, Wv, Wo))

    mode = _detect_mode(attention_mask[0, 0])
    masked = mode == "generic"
    nc = _get_kernel(mode)

    hT = np.ascontiguousarray(hidden_states[0].T).astype(NPBF16)
    cosT = np.ascontiguousarray(cos[0].T).astype(np.float32)
    sinTe = np.ascontiguousarray(sin[0].T).astype(np.float32)
    sinTe[:64] = -sinTe[:64]
    sinTe = np.ascontiguousarray(np.roll(sinTe, 64, axis=0))
    eye = np.eye(128, dtype=NPBF16)
    if mode == "causal":
        jj = np.arange(128)[:, None]
        ii = np.arange(512)[None, :]
        m01 = np.concatenate(
            [(128 * p + jj <= ii).astype(NPBF16) for p in range(4)], axis=0
        )

    in_maps = []
    for c in range(N_CORES):
        # weight DRAM layout: per f-block (q0..q3, k, v): row = partition
        # (contraction row within ktile), col = ktile*128 + out-col
        cols = [Wq[:, (c * QH + h) * HD:(c * QH + h + 1) * HD] for h in range(QH)]
        cols.append(Wk[:, c * HD:(c + 1) * HD])
        cols.append(Wv[:, c * HD:(c + 1) * HD])
        wqkv = np.concatenate(
            [w.reshape(KT, 128, 128).transpose(1, 0, 2).reshape(128, KT * 128)
             for w in cols],
            axis=0,
        ).astype(NPBF16)
        m = {
            "hT": hT,
            "wqkv": wqkv,
            "wo": Wo[c * F:(c + 1) * F, :].astype(NPBF16),
            "cosT": cosT,
            "sinTe": sinTe,
            "eye": eye,
        }
        if masked:
            m["maskT"] = (
                np.ascontiguousarray(attention_mask[0, 0].T).astype(np.float32)
                * math.sqrt(HD)
            )
        if mode == "causal":
            m["m01"] = m01
        in_maps.append(m)

    trace = _collect_exec_info is not None
    res = run_bass_kernel_spmd(nc, in_maps, list(range(N_CORES)), trace=trace)
    if trace:
        _collect_exec_info["exec_time_ns"] = res.exec_time_ns
        _collect_exec_info["results"] = res

    acc = res.results[0]["out"].astype(np.float64)
    for c in range(1, N_CORES):
        acc += res.results[c]["out"].astype(np.float64)
    return acc.astype(np.float32)[None, :, :]


# revision 16
# speedup vs baseline: 1.1089x; 1.0148x over previous
"""Multi-head attention (GQA, 32 q-heads / 8 kv-heads, S=2048, H=4096) on 8
Trainium2 NeuronCores.

Sharding: tensor-parallel across heads. Core c owns kv-head c and q-heads
4c..4c+3 (Wq/Wk/Wv column-sharded, Wo row-sharded). Each core computes a
partial output [S, H]; the host sums the 8 partials.

Per-core dataflow (everything bf16 into the PE, fp32 accumulation):
  A) qT/kT/vT = W.T @ hiddenT  (weights stationary, hiddenT moving),
     f-outer passes (one PSUM bank per 512-col pass, 3-bank rotation),
     RoPE applied straight out of PSUM in the transposed [hd, s] layout,
     vT transposed back to natural v[s, hd] via PE-transpose.
  B) per (i-group, q-head): scoresT[j,i] = kT.T @ qT -> E = exp(scale*s)
     denom via DVE chain-sum of the 16 E tiles + ONE ones-matmul,
     attnT[d,i] = v.T @ E (PSUM), normalized on DVE into aT.
  C) partial_out[s,:] = attnT.T @ Wo_c, interleaved per i-group into
     phase B so phase C's PE work hides phase B's ACT (exp) work.
"""

import math
import os
import sys

if os.path.isdir("/opt/trn_rl_repo") and "/opt/trn_rl_repo" not in sys.path:
    sys.path.insert(0, "/opt/trn_rl_repo")

import numpy as np
import ml_dtypes

import concourse.bacc as bacc
import concourse.mybir as mybir
from concourse import tile
from concourse.bass_utils import run_bass_kernel_spmd

BF16 = mybir.dt.bfloat16
F32 = mybir.dt.float32
NPBF16 = ml_dtypes.bfloat16

S = 2048
H = 4096
HD = 128
NH = 32
NKV = 8
N_CORES = 8
QH = NH // N_CORES          # q-heads per core = 4
F = QH * HD                 # q feature columns per core = 512
KT = H // 128               # contraction tiles for the projections = 32
ST = S // 128               # 128-row tiles along S = 16
SG = S // 512               # 512-wide groups along S = 4
NF = QH + 2                 # projection passes per s-group: q0..q3, v, k
SCALE = 1.0 / math.sqrt(HD)

_BUILT = {}


def _build(mode: str):
    masked = mode == "generic"
    nc = bacc.Bacc(None, target_bir_lowering=False)

    hT = nc.declare_dram_parameter("hT", [H, S], BF16, isOutput=False)
    # weight layout: per 128-col feature chunk f (q0..q3, k, v), row = f*128
    # + partition, col = ktile*128 + out-col (8KB contiguous rows -> one
    # full-rate DMA per f-pass)
    wqkv = nc.declare_dram_parameter("wqkv", [NF * 128, KT * 128], BF16,
                                     isOutput=False)
    wo = nc.declare_dram_parameter("wo", [F, H], BF16, isOutput=False)
    cosT = nc.declare_dram_parameter("cosT", [HD, S], F32, isOutput=False)
    sinTe = nc.declare_dram_parameter("sinTe", [HD, S], F32, isOutput=False)
    eye = nc.declare_dram_parameter("eye", [128, 128], BF16, isOutput=False)
    if masked:
        maskT = nc.declare_dram_parameter("maskT", [S, S], F32, isOutput=False)
    if mode == "causal":
        # four 0/1 diagonal-tile patterns, stacked [4*128, 512]
        m01 = nc.declare_dram_parameter("m01", [4 * 128, 512], BF16, isOutput=False)
    out = nc.declare_dram_parameter("out", [S, H], F32, isOutput=True)

    # matmul pass order within each s-group: q heads, then v, then k.
    # (v before k so the PE v-transposes can hide under the k pass.)
    # sg3 runs k/v early so kT's RoPE is done before phase B's first scores
    # and the v-transposes hide under the q3 pass.
    F_ORDER = [0, 1, 2, 3, 5, 4]  # logical f: 0..3 = q heads, 4 = k, 5 = v
    F_ORDER_LAST = [0, 1, 2, 4, 5, 3]

    with tile.TileContext(nc) as tc:
        with tc.tile_pool(name="persist", bufs=1) as pp:
            cos_sb = pp.tile([HD, S], F32, tag="cos")
            sin_sb = pp.tile([HD, S], F32, tag="sin")
            eye_sb = pp.tile([128, 128], BF16, tag="eye")
            ones_sb = pp.tile([128, 128], BF16, tag="ones")
            qT_sb = [pp.tile([HD, S], BF16, tag=f"qT{h}", name=f"qT{h}") for h in range(QH)]
            kT_sb = pp.tile([HD, S], BF16, tag="kT")
            v_sb = pp.tile([128, ST * HD], BF16, tag="v")
            # aT double-buffered by i-group parity: phase C reads parity p
            # while phase B writes parity 1-p (avoids any WAR coupling)
            aT_sb = [
                [pp.tile([HD, 512], BF16, tag=f"aT{p}_{h}", name=f"aT{p}_{h}")
                 for h in range(QH)]
                for p in range(2)
            ]
            vt_t = pp.tile([128, 512], BF16, tag="vt")
            t1_t = pp.tile([128, 512], F32, tag="t1")
            t2_t = pp.tile([128, 512], F32, tag="t2")
            if mode == "causal":
                m01_sb = pp.tile([128, 4 * 512], BF16, tag="m01")

            nc.gpsimd.memset(ones_sb[:], 1.0)

            # ---------------- Phase A: projections + RoPE ----------------
            with (
                tc.tile_pool(name="phA_w", bufs=1) as paw,
                tc.tile_pool(name="phA_h", bufs=2) as pah,
                tc.tile_pool(name="phA_ps", bufs=3, space="PSUM") as pap,
                tc.tile_pool(name="phA_pst", bufs=1, space="PSUM") as papt,
                tc.tile_pool(name="phA_warm", bufs=1, space="PSUM") as pwarm,
            ):
                w_sb = paw.tile([128, NF * KT * 128], BF16, tag="wqkv")
                # w_view[p, f, k, c]
                w_view = w_sb[:].rearrange("p (f a c) -> p f a c", f=NF, a=KT)
                w_flat = w_sb[:].rearrange("p (f q) -> p f q", f=NF)
                w_src = wqkv[:].rearrange("(f p) q -> p f q", f=NF)
                CH = 4  # hidden ktiles per DMA chunk
                for sg in range(SG):
                    hc = pah.tile([128, KT * 512], BF16, tag="hc")
                    h_view = hc[:].rearrange("p (a s) -> p a s", a=KT)
                    h_src = hT[:, sg * 512:(sg + 1) * 512].rearrange(
                        "(a p) s -> p a s", p=128
                    )
                    if sg == 0:
                        # queue order: w for pass 0, the whole hidden group
                        # (pass 0 is DMA-paced), cos/sin (needed by the first
                        # RoPE), remaining w passes, then eye/m01
                        nc.sync.dma_start(w_flat[:, 0], w_src[:, 0])
                    for lo in range(0, KT, CH):
                        csl = slice(lo, lo + CH)
                        nc.sync.dma_start(h_view[:, csl, :], h_src[:, csl, :])
                    if sg == 0:
                        # remaining w blocks in pass order; cos/sin after (the
                        # first RoPE only gates PSUM-bank reuse at pass 3)
                        for fi in (1, 2, 3, 5, 4):
                            nc.sync.dma_start(w_flat[:, fi], w_src[:, fi])
                        nc.sync.dma_start(cos_sb[:], cosT[:])
                        nc.sync.dma_start(sin_sb[:], sinTe[:])
                        nc.sync.dma_start(eye_sb[:], eye[:])
                        if mode == "causal":
                            nc.sync.dma_start(
                                m01_sb[:].rearrange("p (a i) -> p a i", a=4),
                                m01[:].rearrange("(a p) i -> p a i", p=128),
                            )
                    if sg == 0:
                        # warm the PE p-state while the first weight/hidden
                        # DMAs land (throwaway accumulations into one bank)
                        warm = pwarm.tile([128, 128], F32, tag="warm")
                        for _ in range(40):
                            nc.tensor.matmul(
                                warm[:], ones_sb[:], ones_sb[:],
                                start=True, stop=True,
                            )
                    sl = slice(sg * 512, (sg + 1) * 512)
                    for f in (F_ORDER_LAST if sg == SG - 1 else F_ORDER):
                        ps = pap.tile([128, 512], F32, tag="proj")
                        for k in range(KT):
                            nc.tensor.matmul(
                                ps[:],
                                w_view[:, f, k, :],
                                hc[:, k * 512:(k + 1) * 512],
                                start=(k == 0),
                                stop=(k == KT - 1),
                            )
                        if f == 5:
                            # v: copy PSUM->SBUF (ACT), transposes emitted
                            # after the k pass below
                            nc.scalar.copy(vt_t[:], ps[:])
                        else:
                            # RoPE straight out of PSUM:
                            # dest[d] = ps[d]*cos[d] + ps[(d+64)%128]*sinTe[d]
                            dest = (qT_sb[f] if f < QH else kT_sb)[:, sl]
                            nc.vector.tensor_mul(t1_t[:], ps[:], cos_sb[:, sl])
                            nc.vector.tensor_mul(
                                t2_t[0:64, :], ps[64:128, :], sin_sb[64:128, sl]
                            )
                            nc.vector.tensor_mul(
                                t2_t[64:128, :], ps[0:64, :], sin_sb[0:64, sl]
                            )
                            nc.vector.tensor_add(dest, t1_t[:], t2_t[:])
                    # v transposes (hidden under the k pass's PE stream)
                    for b in range(4):
                        jt = sg * 4 + b
                        pst = papt.tile([128, 128], BF16, tag="vtr")
                        nc.tensor.transpose(
                            pst[:], vt_t[:, b * 128:(b + 1) * 128], eye_sb[:]
                        )
                        nc.scalar.copy(v_sb[:, jt * HD:(jt + 1) * HD], pst[:])

            # ---------------- Phase B + C interleaved ----------------
            with tc.tile_pool(name="late", bufs=1) as pl:
              wo_sb = pl.tile([128, QH * H], BF16, tag="wo")
              nc.sync.dma_start(
                  wo_sb[:].rearrange("p (a o) -> p a o", a=QH),
                  wo[:].rearrange("(a p) o -> p a o", p=128),
              )
              with (
                tc.tile_pool(name="phB_E", bufs=20) as pe_pool,
                tc.tile_pool(name="phB_acc", bufs=3) as pacc,
                tc.tile_pool(name="phB_rc", bufs=3) as prc,
                tc.tile_pool(name="phB_m", bufs=3) as pbm,
                tc.tile_pool(name="phB_s", bufs=4, space="PSUM") as pbs,
                tc.tile_pool(name="phB_pv", bufs=1, space="PSUM") as pbv,
                tc.tile_pool(name="phB_den", bufs=1, space="PSUM") as pbd,
                tc.tile_pool(name="phC_ps", bufs=2, space="PSUM") as pcp,
                tc.tile_pool(name="phC_o", bufs=4) as pco,
              ):
                  def c_block(st):
                      ssl = slice(st * 128, (st + 1) * 128)
                      par = (st // 4) % 2
                      off = (st % 4) * 128
                      for ho in range(H // 512):
                          po = pcp.tile([128, 512], F32, tag="o")
                          for f4 in range(QH):
                              nc.tensor.matmul(
                                  po[:],
                                  aT_sb[par][f4][:, off:off + 128],
                                  wo_sb[:, f4 * H + ho * 512:f4 * H + (ho + 1) * 512],
                                  start=(f4 == 0),
                                  stop=(f4 == QH - 1),
                              )
                          ob = pco.tile([128, 512], F32, tag="ob")
                          if ho % 2 == 0:
                              nc.scalar.copy(ob[:], po[:])
                          else:
                              nc.vector.tensor_copy(ob[:], po[:])
                          nc.sync.dma_start(
                              out[ssl, ho * 512:(ho + 1) * 512], ob[:]
                          )

                  for ig in range(SG):
                      isl = slice(ig * 512, (ig + 1) * 512)
                      par = ig % 2
                      jts = list(range(4 * ig + 4)) if mode == "causal" else list(range(ST))
                      for h in range(QH):
                          qh = qT_sb[h]
                          et = [
                              pe_pool.tile([128, 512], BF16, tag="E",
                                           name=f"E{ig}_{h}_{j}")
                              for j in jts
                          ]
                          for x, jt in enumerate(jts):
                              sps = pbs.tile([128, 512], F32, tag="s")
                              nc.tensor.matmul(
                                  sps[:],
                                  kT_sb[:, jt * 128:(jt + 1) * 128],
                                  qh[:, isl],
                                  start=True,
                                  stop=True,
                              )
                              if masked:
                                  mt = pbm.tile([128, 512], F32, tag="mT")
                                  nc.sync.dma_start(
                                      mt[:], maskT[jt * 128:(jt + 1) * 128, isl]
                                  )
                                  sm = pbm.tile([128, 512], F32, tag="sm")
                                  nc.vector.tensor_add(sm[:], sps[:], mt[:])
                                  nc.scalar.activation(
                                      et[x][:], sm[:],
                                      mybir.ActivationFunctionType.Exp,
                                      scale=SCALE,
                                  )
                              elif mode == "causal" and jt >= 4 * ig:
                                  p = jt - 4 * ig
                                  etmp = pbm.tile([128, 512], BF16, tag="etmp")
                                  nc.scalar.activation(
                                      etmp[:], sps[:],
                                      mybir.ActivationFunctionType.Exp,
                                      scale=SCALE,
                                  )
                                  nc.vector.tensor_mul(
                                      et[x][:], etmp[:],
                                      m01_sb[:, p * 512:(p + 1) * 512],
                                  )
                              else:
                                  nc.scalar.activation(
                                      et[x][:], sps[:],
                                      mybir.ActivationFunctionType.Exp,
                                      scale=SCALE,
                                  )
                          # denominator: chain-sum the E tiles on DVE (bf16),
                          # finish with a single ones-matmul
                          acc = pacc.tile([128, 512], BF16, tag="acc")
                          nc.vector.tensor_add(acc[:], et[0][:], et[1][:])
                          for x in range(2, len(jts)):
                              nxt = pacc.tile([128, 512], BF16, tag="acc")
                              nc.vector.tensor_add(nxt[:], acc[:], et[x][:])
                              acc = nxt
                          pv = pbv.tile([128, 512], F32, tag="pv")
                          for x, jt in enumerate(jts):
                              nc.tensor.matmul(
                                  pv[:], v_sb[:, jt * HD:(jt + 1) * HD], et[x][:],
                                  start=(x == 0), stop=(x == len(jts) - 1),
                              )
                          den = pbd.tile([128, 512], F32, tag="den")
                          nc.tensor.matmul(
                              den[:], ones_sb[:], acc[:], start=True, stop=True
                          )
                          rc = prc.tile([128, 512], F32, tag="rc")
                          nc.vector.reciprocal_approx_fast(rc[:], den[:])
                          nc.vector.tensor_mul(aT_sb[par][h][:], pv[:], rc[:])
                          if ig >= 1:
                              c_block(4 * (ig - 1) + h)
                  for st in range(4 * (SG - 1), ST):
                      c_block(st)

    nc.finalize()
    return nc


def _get_kernel(mode: str):
    if mode not in _BUILT:
        _BUILT[mode] = _build(mode)
    return _BUILT[mode]


def _detect_mode(mask2d):
    if not np.any(mask2d):
        return "nomask"
    neg = mask2d[0, 1]
    if neg <= -1e4 and np.array_equal(
        mask2d, np.triu(np.full((S, S), neg, mask2d.dtype), k=1)
    ):
        return "causal"
    return "generic"


def kernel(hidden_states, position_ids, attention_mask, cos, sin, Wq, Wk, Wv, Wo,
           _collect_exec_info=None):
    hidden_states = np.asarray(hidden_states)
    attention_mask = np.asarray(attention_mask)
    cos = np.asarray(cos)
    sin = np.asarray(sin)
    Wq, Wk, Wv, Wo = (np.asarray(a) for a in (Wq, Wk, Wv, Wo))

    mode = _detect_mode(attention_mask[0, 0])
    masked = mode == "generic"
    nc = _get_kernel(mode)

    hT = np.ascontiguousarray(hidden_states[0].T).astype(NPBF16)
    cosT = np.ascontiguousarray(cos[0].T).astype(np.float32)
    sinTe = np.ascontiguousarray(sin[0].T).astype(np.float32)
    sinTe[:64] = -sinTe[:64]
    sinTe = np.ascontiguousarray(np.roll(sinTe, 64, axis=0))
    eye = np.eye(128, dtype=NPBF16)
    if mode == "causal":
        jj = np.arange(128)[:, None]
        ii = np.arange(512)[None, :]
        m01 = np.concatenate(
            [(128 * p + jj <= ii).astype(NPBF16) for p in range(4)], axis=0
        )

    in_maps = []
    for c in range(N_CORES):
        # weight DRAM layout: per f-block (q0..q3, k, v): row = partition
        # (contraction row within ktile), col = ktile*128 + out-col
        cols = [Wq[:, (c * QH + h) * HD:(c * QH + h + 1) * HD] for h in range(QH)]
        cols.append(Wk[:, c * HD:(c + 1) * HD])
        cols.append(Wv[:, c * HD:(c + 1) * HD])
        wqkv = np.concatenate(
            [w.reshape(KT, 128, 128).transpose(1, 0, 2).reshape(128, KT * 128)
             for w in cols],
            axis=0,
        ).astype(NPBF16)
        m = {
            "hT": hT,
            "wqkv": wqkv,
            "wo": Wo[c * F:(c + 1) * F, :].astype(NPBF16),
            "cosT": cosT,
            "sinTe": sinTe,
            "eye": eye,
        }
        if masked:
            m["maskT"] = (
                np.ascontiguousarray(attention_mask[0, 0].T).astype(np.float32)
                * math.sqrt(HD)
            )
        if mode == "causal":
            m["m01"] = m01
        in_maps.append(m)

    trace = _collect_exec_info is not None
    res = run_bass_kernel_spmd(nc, in_maps, list(range(N_CORES)), trace=trace)
    if trace:
        _collect_exec_info["exec_time_ns"] = res.exec_time_ns
        _collect_exec_info["results"] = res

    acc = res.results[0]["out"].astype(np.float64)
    for c in range(1, N_CORES):
        acc += res.results[c]["out"].astype(np.float64)
    return acc.astype(np.float32)[None, :, :]
